# revision 1
# baseline (speedup 1.0000x reference)
"""Trainium2 Bass kernel for nn_CoconAttention (dense transformer attention block).

Sharding: 8 cores = 4 batches x 2 head-groups (8 heads each). Each core gets
pre-transposed/sliced inputs, computes its partial output outT [1024, 896]
(transposed, pre-b_proj), and the host sums head-group pairs + transposes.

On-device layout (per core, H=8 heads, Dh=64, T=896, Tc=128, S=1024):
  qT, kT   : feature-major [64*2, tok] pair tiles  (fp32, fp32r matmuls)
  scores^T : [128 keys, tok] per key-chunk c, psum  (fp32r MMs, exp on ACT)
  probs^T  : bf16, masked via precomputed band masks (DVE mult)
  v_aug    : [keys, 8 heads, 65] bf16 (last col = ones -> denominator row in PV psum)
  a^T      : [512, tok] fp32, normalized by 1/denom (DVE), then proj via fp32r MMs
"""
import os
import sys

import numpy as np
import ml_dtypes

try:
    import concourse.bass as bass
except ImportError:  # fresh grading dir: fall back to the repo location
    sys.path.insert(0, "/opt/trn_rl_repo")
    import concourse.bass as bass
import concourse.bacc as bacc

import concourse.tile as tile
from concourse import mybir
from concourse.bass_utils import run_bass_kernel_spmd
from contextlib import ExitStack

F32 = mybir.dt.float32
BF16 = mybir.dt.bfloat16
F32R = mybir.dt.float32r
AF = mybir.ActivationFunctionType
ALU = mybir.AluOpType

T, Tc, NX = 896, 128, 1024
TCH = ((0, 512), (512, 896))  # tok chunks
NPAIR = 4  # head pairs per core


def _r(ap):
    return ap.bitcast(F32R)


def _bc0(ap, n):
    """Partition-broadcast read AP: [1, ...] -> [n, ...] with partition step 0."""
    return bass.AP(tensor=ap.tensor, offset=ap.offset, ap=[[0, n]] + list(ap.ap[1:]))


def _rect(c, ts, te):
    """Live (unmasked) column range of scores chunk c within tok range [ts,te)."""
    cs = max(max(0, 128 * (c - 1)), ts)
    return None if cs >= te else (cs, te)


def _band_pieces(c, ts, te):
    """Mask applications for chunk c in [ts,te): (s0, e0, mask_col_offset)."""
    if c == 0:
        bs, be, moff, borig = 0, 128, 128, 0  # diag half only
    elif c <= 6:
        bs = 128 * (c - 1)
        be, moff, borig = bs + 256, 0, bs  # causal(128) + diag(128)
    else:
        bs, be, moff, borig = 768, 896, 0, 768  # causal half only
    s0, e0 = max(bs, ts), min(be, te)
    if s0 >= e0:
        return []
    return [(s0, e0, moff + (s0 - borig))]


def build_nc():
    nc = bacc.Bacc("TRN2", target_bir_lowering=False)

    x_h = nc.dram_tensor("xT", [NX, T], F32, kind="ExternalInput")
    ctx_h = nc.dram_tensor("ctxT", [NX, Tc], BF16, kind="ExternalInput")
    wq_h = nc.dram_tensor("w_q", [NX, 512], F32, kind="ExternalInput")
    wk_h = nc.dram_tensor("w_k", [NX, 512], F32, kind="ExternalInput")
    wv_h = nc.dram_tensor("w_v", [NX, 512], F32, kind="ExternalInput")
    wkc_h = nc.dram_tensor("w_kc", [NX, 512], BF16, kind="ExternalInput")
    wvc_h = nc.dram_tensor("w_vc", [NX, 512], BF16, kind="ExternalInput")
    wpj_h = nc.dram_tensor("w_pj", [512, NX], F32, kind="ExternalInput")
    bqk_h = nc.dram_tensor("b_qk", [128, 8], F32, kind="ExternalInput")
    bkc_h = nc.dram_tensor("b_kc", [128, 4], F32, kind="ExternalInput")
    bv_h = nc.dram_tensor("b_v", [1, 512], F32, kind="ExternalInput")
    bvc_h = nc.dram_tensor("b_vc", [1, 512], F32, kind="ExternalInput")
    mb_h = nc.dram_tensor("mband", [128, 256], BF16, kind="ExternalInput")
    out_h = nc.dram_tensor("outT", [NX, T], F32, kind="ExternalOutput")

    with tile.TileContext(nc) as tc, ExitStack() as top:
        consts = top.enter_context(tc.tile_pool(name="consts", bufs=1))
        qkp = top.enter_context(tc.tile_pool(name="qkp", bufs=1))
        vtp = top.enter_context(tc.tile_pool(name="vtp", bufs=1))
        atp = top.enter_context(tc.tile_pool(name="atp", bufs=1))
        misc = top.enter_context(tc.tile_pool(name="misc", bufs=2))
        probsp = top.enter_context(tc.tile_pool(name="probsp", bufs=4))
        outp = top.enter_context(tc.tile_pool(name="outp", bufs=2))
        dramp = top.enter_context(tc.tile_pool(name="dramp", bufs=2, space="DRAM"))

        # ---- constants ----
        maskband = consts.tile([128, 256], BF16, name="maskband")
        nc.sync.dma_start(out=maskband, in_=mb_h[:, :])

        ebias = consts.tile([128, 2], F32, name="ebias")  # exp bias consts: [0]=0, [1]=ctx -2
        nc.vector.memset(ebias[:, 0:1], 0.0)
        nc.vector.memset(ebias[:, 1:2], -2.0)

        bias_qk = consts.tile([128, 8], F32, name="bias_qk")
        nc.sync.dma_start(out=bias_qk, in_=bqk_h[:, :])
        bias_kc = consts.tile([128, 4], F32, name="bias_kc")
        nc.sync.dma_start(out=bias_kc, in_=bkc_h[:, :])
        bvb = consts.tile([128, 512], F32, name="bvb")
        nc.gpsimd.dma_start(out=bvb, in_=_bc0(bv_h[:, :], 128))
        bvcb = consts.tile([128, 512], F32, name="bvcb")
        nc.gpsimd.dma_start(out=bvcb, in_=_bc0(bvc_h[:, :], 128))

        # ---- persistent activation tiles ----
        qT = [qkp.tile([128, T], F32R, name=f"qT{p}") for p in range(NPAIR)]
        kT = [qkp.tile([128, Tc + T], F32R, name=f"kT{p}") for p in range(NPAIR)]
        v_sb = [vtp.tile([128, 8, 65], BF16, name=f"v{c}") for c in range(8)]
        for c in range(8):
            nc.vector.memset(v_sb[c][:, :, 64:65], 1.0)
        aT = [atp.tile([128, T], F32R, name=f"aT{p}") for p in range(NPAIR)]


        with ExitStack() as ph1:
            wts = ph1.enter_context(tc.tile_pool(name="wts", bufs=1))
            xp = ph1.enter_context(tc.tile_pool(name="xp", bufs=1))
            pps = ph1.enter_context(tc.tile_pool(name="pps", bufs=2, space="PSUM"))
            scp = ph1.enter_context(tc.tile_pool(name="scp", bufs=2, space="PSUM"))
            atps = ph1.enter_context(tc.tile_pool(name="atps", bufs=2, space="PSUM"))

            # ---- input loads (small bf16 ctx operands first, then per-k-chunk) ----
            ctx_sb = wts.tile([128, 8, Tc], BF16, name="ctx_sb")
            nc.sync.dma_start(out=ctx_sb, in_=ctx_h[:, :].rearrange("(kc p) t -> p kc t", p=128))
            wkc_sb = wts.tile([128, 8, 512], BF16, name="wkc_sb")
            nc.sync.dma_start(out=wkc_sb, in_=wkc_h[:, :].rearrange("(kc p) f -> p kc f", p=128))
            wvc_sb = wts.tile([128, 8, 512], BF16, name="wvc_sb")
            nc.sync.dma_start(out=wvc_sb, in_=wvc_h[:, :].rearrange("(kc p) f -> p kc f", p=128))
            x_sb = xp.tile([128, 8, T], F32R, name="x_sb")
            wq_sb = wts.tile([128, 8, 512], F32R, name="wq_sb")
            wk_sb = wts.tile([128, 8, 512], F32R, name="wk_sb")
            wv_sb = wts.tile([128, 8, 512], F32R, name="wv_sb")
            xr = x_h[:, :].rearrange("(kc p) t -> p kc t", p=128).bitcast(F32R)
            qr = wq_h[:, :].rearrange("(kc p) f -> p kc f", p=128).bitcast(F32R)
            kr = wk_h[:, :].rearrange("(kc p) f -> p kc f", p=128).bitcast(F32R)
            vr = wv_h[:, :].rearrange("(kc p) f -> p kc f", p=128).bitcast(F32R)
            for kc in range(8):
                nc.sync.dma_start(out=x_sb[:, kc, :], in_=xr[:, kc, :])
                nc.sync.dma_start(out=wv_sb[:, kc, :], in_=vr[:, kc, :])
            for kc in range(8):
                nc.sync.dma_start(out=wq_sb[:, kc, :], in_=qr[:, kc, :])
                nc.sync.dma_start(out=wk_sb[:, kc, :], in_=kr[:, kc, :])

            # ---- ctx projections (bf16): kcT + vc ----
            for f in range(4):
                pt = pps.tile([128, 512], F32, tag="pp", name=f"pkc{f}")
                for kc in range(8):
                    nc.tensor.matmul(
                        pt[:, 0:Tc], wkc_sb[:, kc, 128 * f:128 * f + 128],
                        ctx_sb[:, kc, :], start=(kc == 0), stop=(kc == 7))
                nc.scalar.activation(
                    out=kT[f][:, 0:Tc], in_=pt[:, 0:Tc], func=AF.Identity,
                    bias=bias_kc[:, f:f + 1], scale=1.0)
            pt = pps.tile([128, 512], F32, tag="pp", name="pvc")
            for kc in range(8):
                nc.tensor.matmul(
                    pt[:, 0:512], ctx_sb[:, kc, :], wvc_sb[:, kc, :],
                    start=(kc == 0), stop=(kc == 7))
            nc.vector.tensor_add(
                out=v_sb[0][:, :, 0:64],
                in0=pt[:, 0:512].rearrange("p (h d) -> p h d", h=8),
                in1=bvcb.rearrange("p (h d) -> p h d", h=8))

            # ---- v projection (natural layout, fp32r) ----
            for tt in range(7):
                pt = pps.tile([128, 512], F32, tag="pp", name=f"pv{tt}")
                for kc in range(8):
                    nc.tensor.matmul(
                        pt[:, 0:512], x_sb[:, kc, 128 * tt:128 * tt + 128],
                        wv_sb[:, kc, :], start=(kc == 0), stop=(kc == 7))
                nc.vector.tensor_add(
                    out=v_sb[1 + tt][:, :, 0:64],
                    in0=pt[:, 0:512].rearrange("p (h d) -> p h d", h=8),
                    in1=bvb.rearrange("p (h d) -> p h d", h=8))

            # ---- qT / kT projections (transposed layout, fp32r), pair-ordered ----
            def qk_ftile(w_sb, f, dest, dcol, bias_col):
                for ts, te in TCH:
                    pt = pps.tile([128, 512], F32, tag="pp",
                                  name=f"pqk{bias_col}{ts}")
                    for kc in range(8):
                        nc.tensor.matmul(
                            pt[:, 0:te - ts], w_sb[:, kc, 128 * f:128 * f + 128],
                            x_sb[:, kc, ts:te], start=(kc == 0), stop=(kc == 7))
                    nc.scalar.activation(
                        out=dest[:, dcol + ts:dcol + te], in_=pt[:, 0:te - ts],
                        func=AF.Identity, bias=bias_qk[:, bias_col:bias_col + 1],
                        scale=1.0)

            for p in range(NPAIR):
                qk_ftile(wq_sb, p, qT[p], 0, p)
                qk_ftile(wk_sb, p, kT[p], Tc, 4 + p)

            # ---- attention, per head-pair ----
            for p in range(NPAIR):
                tmpa = misc.tile([128, T], F32, tag="tmpa", name=f"tmpa{p}")
                tmpb = misc.tile([128, T], F32, tag="tmpb", name=f"tmpb{p}")
                dm = misc.tile([128, 2, 7], F32, tag="dm", name=f"dm{p}")
                rdm = misc.tile([128, 2, 7], F32, tag="rdm", name=f"rdm{p}")
                rscr = dramp.tile([2, T], F32, tag="rscr", name=f"rscr{p}")
                rbc = misc.tile([128, T], F32, tag="rbc", name=f"rbc{p}")
                for t_i, (ts, te) in enumerate(TCH):
                    last_c = 4 if t_i == 0 else 7
                    at_ps = [atps.tile([65, 512], F32, tag="atp", name=f"at{p}{t_i}{hi}")
                             for hi in range(2)]
                    for c in range(8):
                        rr = _rect(c, ts, te)
                        if rr is None:
                            continue
                        cs, _ = rr
                        mmcs = cs if te - cs >= 256 else max(ts, te - 256)
                        sc = scp.tile([128, 2, 512], F32, tag="sc", name=f"sc{p}{t_i}{c}")
                        for hi in range(2):
                            nc.tensor.matmul(
                                sc[:, hi, mmcs - ts:te - ts],
                                kT[p][64 * hi:64 * hi + 64, 128 * c:128 * c + 128],
                                qT[p][64 * hi:64 * hi + 64, mmcs:te],
                                start=True, stop=True, tile_position=(64 * hi, 0))
                        pb = probsp.tile([128, 2, 512], BF16, tag="pb", name=f"pb{p}{t_i}{c}")
                        nc.scalar.activation(
                            out=pb[:, :, cs - ts:te - ts], in_=sc[:, :, cs - ts:te - ts],
                            func=AF.Exp,
                            bias=(ebias[:, 1:2] if c == 0 else ebias[:, 0:1]),
                            scale=0.125)
                        for hi in range(2):
                            for s0, e0, mc in _band_pieces(c, ts, te):
                                nc.vector.tensor_mul(
                                    out=pb[:, hi, s0 - ts:e0 - ts],
                                    in0=pb[:, hi, s0 - ts:e0 - ts],
                                    in1=maskband[:, mc:mc + (e0 - s0)])
                        for hi in range(2):
                            nc.tensor.matmul(
                                at_ps[hi][0:65, cs - ts:te - ts],
                                v_sb[c][:, 2 * p + hi, :],
                                pb[:, hi, cs - ts:te - ts],
                                start=(c == 0), stop=(c == last_c),
                                skip_group_check=True)
                    # copy aT(+denom row 64) out of psum
                    nc.vector.tensor_copy(out=tmpa[0:65, ts:te], in_=at_ps[0][0:65, 0:te - ts])
                    nc.vector.tensor_copy(out=tmpb[0:65, ts:te], in_=at_ps[1][0:65, 0:te - ts])
                # assemble pair: aT rows, denominators -> reciprocal -> broadcast
                nc.gpsimd.dma_start(out=aT[p][0:64, :], in_=tmpa[0:64, :].bitcast(F32R))
                nc.gpsimd.dma_start(out=aT[p][64:128, :], in_=tmpb[0:64, :].bitcast(F32R))
                nc.gpsimd.dma_start(out=dm[:, 0, :], in_=tmpa[64:65, 0:T])
                nc.gpsimd.dma_start(out=dm[:, 1, :], in_=tmpb[64:65, 0:T])
                nc.vector.reciprocal(out=rdm, in_=dm)
                for hi in range(2):
                    nc.gpsimd.dma_start(out=rscr[hi:hi + 1, :], in_=rdm[:, hi, :])
                    nc.gpsimd.dma_start(
                        out=rbc[64 * hi:64 * hi + 64, :], in_=_bc0(rscr[hi:hi + 1, :], 64))
                nc.vector.tensor_mul(out=aT[p], in0=aT[p], in1=rbc)

        # ---- output projection (fp32r) ----
        with tc.tile_pool(name="w2", bufs=1) as w2, \
                tc.tile_pool(name="ops", bufs=3, space="PSUM") as ops:
            wpj_sb = w2.tile([128, 4, 1024], F32R, name="wpj_sb")
            pr = wpj_h[:, :].rearrange("(kc p) o -> p kc o", p=128).bitcast(F32R)
            for kc in range(4):
                nc.sync.dma_start(out=wpj_sb[:, kc, :], in_=pr[:, kc, :])
            for of in range(8):
                pt = ops.tile([128, 1024], F32, tag="ops", name=f"po{of}")
                for ts, te in TCH:
                    for kc in range(4):
                        nc.tensor.matmul(
                            pt[:, ts:te], wpj_sb[:, kc, 128 * of:128 * of + 128],
                            aT[kc][:, ts:te], start=(kc == 0), stop=(kc == 3))
                ob = outp.tile([128, T], F32, tag="ob", name=f"ob{of}")
                nc.scalar.copy(out=ob, in_=pt[:, 0:T])
                nc.sync.dma_start(out=out_h[128 * of:128 * of + 128, :], in_=ob)

    if not nc.is_finalized():
        nc.finalize()
    return nc


_NC_CACHE = {}


def _get_nc():
    if "nc" not in _NC_CACHE:
        _NC_CACHE["nc"] = build_nc()
    return _NC_CACHE["nc"]


def _pack128(v):
    """[128*n] -> [128, n] with [p, f] = v[128*f + p]."""
    n = v.shape[0] // 128
    return np.ascontiguousarray(v.reshape(n, 128).T)


def make_in_maps(inputs):
    bf16 = ml_dtypes.bfloat16
    x = np.asarray(inputs["x"], np.float32)
    ctx_seq = np.asarray(inputs["context_seq"], np.float32)
    w_ref = np.asarray(inputs["w_ref"], np.float32)
    b_ref = np.asarray(inputs["b_ref"], np.float32)
    w_attn = np.asarray(inputs["w_attn"], np.float32)
    b_attn = np.asarray(inputs["b_attn"], np.float32)
    w_proj = np.asarray(inputs["w_proj"], np.float32)

    # mask band constant: cols 0-127 causal (1 where q>=p), cols 128-255
    # anti-diagonal (0 where q==p else 1)
    qq = np.arange(128)[None, :]
    pp = np.arange(128)[:, None]
    mband = np.concatenate([(qq >= pp), (qq != pp)], axis=1).astype(bf16)
    mband = np.ascontiguousarray(mband)

    in_maps = []
    for b in range(4):
        xT = np.ascontiguousarray(x[b].T)
        ctxT = np.ascontiguousarray(ctx_seq[b].T.astype(bf16))
        for g in range(2):
            sl = slice(512 * g, 512 * g + 512)
            in_maps.append(dict(
                xT=xT,
                ctxT=ctxT,
                w_q=np.ascontiguousarray(w_attn[:, 0 * NX:1 * NX][:, sl]),
                w_k=np.ascontiguousarray(w_attn[:, 1 * NX:2 * NX][:, sl]),
                w_v=np.ascontiguousarray(w_attn[:, 2 * NX:3 * NX][:, sl]),
                w_kc=np.ascontiguousarray(w_ref[:, 0 * NX:1 * NX][:, sl].astype(bf16)),
                w_vc=np.ascontiguousarray(w_ref[:, 1 * NX:2 * NX][:, sl].astype(bf16)),
                w_pj=np.ascontiguousarray(w_proj[sl, :]),
                b_qk=_pack128(np.concatenate([b_attn[0 * NX:1 * NX][sl],
                                              b_attn[1 * NX:2 * NX][sl]])),
                b_kc=_pack128(b_ref[0 * NX:1 * NX][sl]),
                b_v=np.ascontiguousarray(b_attn[2 * NX:3 * NX][sl].reshape(1, 512)),
                b_vc=np.ascontiguousarray(b_ref[1 * NX:2 * NX][sl].reshape(1, 512)),
                mband=mband,
            ))
    return in_maps


def kernel(**inputs):
    b_proj = np.asarray(inputs["b_proj"], np.float32)
    in_maps = make_in_maps(inputs)
    nc = _get_nc()
    res = run_bass_kernel_spmd(nc, in_maps, core_ids=list(range(8)),
                               trace=os.environ.get("COCON_TRACE", "") == "1")
    outs = res.results
    out = np.empty((4, T, NX), np.float32)
    for b in range(4):
        acc = outs[2 * b]["outT"] + outs[2 * b + 1]["outT"]  # [1024, 896]
        out[b] = acc.T + b_proj[None, :]
    if res.exec_time_ns is not None:
        kernel.last_exec_time_ns = res.exec_time_ns
    return out


kernel.last_exec_time_ns = None

